# revision 1
# baseline (speedup 1.0000x reference)
"""Trainium2 Bass kernel for per-sample-LoRA causal self-attention (non-causal SDPA).

Sharding: 8 cores = (batch b in 0..3) x (channel-half in 0..1).
Each core computes q/k/v for its 1024 output channels (8 heads) of sample b,
runs attention for those heads, and produces a partial output projection
(contraction over its half of the y channels). Host sums the two partials
per sample and transposes back.

All matmuls run as float32r (TF32-like, full PE rate, ~1e-4 rel accuracy).
"""

import os
import sys

sys.path.insert(0, "/opt/trn_rl_repo")

import numpy as np

import concourse.bass as bass  # noqa: F401
import concourse.mybir as mybir
import concourse.tile as tile
from concourse import bacc, bass_utils

F32 = mybir.dt.float32
F32R = mybir.dt.float32r
AF = mybir.ActivationFunctionType

B, T, C = 4, 1024, 2048
H, D, R = 16, 128, 16
HALF = C // 2          # output channels per core
HH = HALF // D         # heads per core = 8
CT = C // 128          # contraction tiles over C = 16
IT = HALF // 128       # contraction tiles over half = 8
CH = 512               # t/free chunk
NCH = T // CH          # = 2
PTP = 2                # s_tiles per pT part
SCALE = 1.0 / float(np.sqrt(D))
ROPE_BASE = 10000.0

_compiled = {}
last_result = None     # BassKernelResults of the most recent run (for test harness)
PHASES = []            # (label, first instruction number) build-time markers


def _mark(nc, label):
    PHASES.append((label, int(nc.get_next_instruction_name().split("-")[1])))


def _build_nc():
    nc = bacc.Bacc("TRN2", target_bir_lowering=False, debug=False, num_devices=8)

    xT = nc.dram_tensor("xT", [C, T], F32R, kind="ExternalInput").ap()
    WqT = nc.dram_tensor("WqT", [C, HALF], F32R, kind="ExternalInput").ap()
    WkT = nc.dram_tensor("WkT", [C, HALF], F32R, kind="ExternalInput").ap()
    WvT = nc.dram_tensor("WvT", [C, HALF], F32R, kind="ExternalInput").ap()
    WoT = nc.dram_tensor("WoT", [HALF, C], F32R, kind="ExternalInput").ap()
    dAT = nc.dram_tensor("dAT", [C, 80], F32R, kind="ExternalInput").ap()
    dBp = nc.dram_tensor("dBp", [80, HALF], F32R, kind="ExternalInput").ap()
    doAT = nc.dram_tensor("doAT", [HALF, R], F32R, kind="ExternalInput").ap()
    doB = nc.dram_tensor("doB", [R, C], F32R, kind="ExternalInput").ap()
    cosT = nc.dram_tensor("cosT", [D, T], F32, kind="ExternalInput").ap()
    sinTs = nc.dram_tensor("sinTs", [D, T], F32, kind="ExternalInput").ap()
    outT = nc.dram_tensor("outT", [C, T], F32, kind="ExternalOutput").ap()
    y_spill = nc.dram_tensor("y_spill", [HH, D, T], F32R).ap()

    with tile.TileContext(nc) as tc:
        with tc.tile_pool(name="tabs", bufs=1) as tabs, \
             tc.tile_pool(name="ps_acc", bufs=2, space="PSUM") as ps_acc, \
             tc.tile_pool(name="ps_s", bufs=2, space="PSUM") as ps_s, \
             tc.tile_pool(name="ps_y", bufs=2, space="PSUM") as ps_y, \
             tc.tile_pool(name="ps_lb", bufs=2, space="PSUM") as ps_lb:

            _mark(nc, 'init')
            # ---------------- resident tables ----------------
            u_sb = tabs.tile([80, T], F32R)
            v_sb = tabs.tile([128, IT, HALF], F32R)   # [t_in_tile, t_tile, vo]
            cos_sb = tabs.tile([D, T], F32)
            sin_sb = tabs.tile([D, T], F32)
            const_f = tabs.tile([128, 129], F32)
            const_r = tabs.tile([128, 129], F32R)
            dB_sb = tabs.tile([80, HALF], F32R)
            doAT_sb = tabs.tile([128, IT, R], F32R)
            uo_acc = tabs.tile([R, T], F32)
            ones128 = const_r[:, 0:1]
            ones1 = const_r[0:1, 1:129]

            # ============ phase 1: u, v, per-head qk+attention ============
            with tc.tile_pool(name="xpool", bufs=1) as xpool:
                x_sb = xpool.tile([128, CT, T], F32R)

                # ---- u = [dqA;dkA;dvA] @ x^T  (rows padded to 0/32/64) ----
                _mark(nc, 'u')
                with tc.tile_pool(name="dap", bufs=1) as dap:
                    dAT_sb = dap.tile([128, CT, 80], F32R)
                    nc.sync.dma_start(dAT_sb[:], dAT.rearrange("(ct p) r -> p ct r", p=128))
                    xr = xT.rearrange("(ct p) t -> p ct t", p=128)
                    for xg in range(8):
                        nc.sync.dma_start(x_sb[:, 2 * xg:2 * xg + 2, :],
                                          xr[:, 2 * xg:2 * xg + 2, :])
                    nc.sync.dma_start(dB_sb[:], dBp[:])
                    nc.sync.dma_start(cos_sb[:], cosT[:])
                    nc.sync.dma_start(sin_sb[:], sinTs[:])
                    nc.sync.dma_start(doAT_sb[:], doAT.rearrange("(it p) r -> p it r", p=128))
                    nc.gpsimd.memset(const_f[:], 1.0)
                    nc.vector.tensor_copy(const_r[:], const_f[:])
                    for ci in range(NCH):
                        ps_u = ps_lb.tile([80, CH], F32, tag="lb")
                        for ct in range(CT):
                            nc.tensor.matmul(ps_u[:], dAT_sb[:, ct, :],
                                             x_sb[:, ct, ci * CH:(ci + 1) * CH],
                                             start=(ct == 0), stop=(ct == CT - 1))
                        nc.scalar.activation(u_sb[:, ci * CH:(ci + 1) * CH], ps_u[:], AF.Copy)

                # ---- P1-v : v natural [t, vo] ----
                _mark(nc, 'v')
                with tc.tile_pool(name="wv", bufs=2) as wvp:
                    for ci in range(2):                # vo chunk of 512
                        wv_half = wvp.tile([128, CT, CH], F32R, tag="wv")
                        wvr = WvT.rearrange("(ct p) o -> p ct o", p=128)
                        for wg in range(4):
                            nc.sync.dma_start(
                                wv_half[:, 4 * wg:4 * wg + 4, :],
                                wvr[:, 4 * wg:4 * wg + 4, ci * CH:(ci + 1) * CH])
                        for tt in range(IT):
                            ps = ps_acc.tile([128, CH], F32, tag="acc")
                            for ct in range(CT):
                                nc.tensor.matmul(ps[:], x_sb[:, ct, tt * 128:(tt + 1) * 128],
                                                 wv_half[:, ct, :],
                                                 start=(ct == 0), stop=False)
                            nc.tensor.matmul(ps[:], u_sb[64:80, tt * 128:(tt + 1) * 128],
                                             dB_sb[64:80, ci * CH:(ci + 1) * CH],
                                             start=False, stop=True)
                            nc.scalar.activation(v_sb[:, tt, ci * CH:(ci + 1) * CH], ps[:],
                                                 AF.Copy)

                # ---- per-head: P1-qk + RoPE + attention ----
                with tc.tile_pool(name="wqk", bufs=4) as wqkp, \
                     tc.tile_pool(name="rope", bufs=2) as rope, \
                     tc.tile_pool(name="qk", bufs=3) as qkp, \
                     tc.tile_pool(name="ptp", bufs=5) as ptp, \
                     tc.tile_pool(name="att", bufs=1) as att:
                    for h in range(HH):
                        _mark(nc, f'qk{h}')
                        rots = []
                        for pi, (wT, urow) in enumerate(((WqT, 0), (WkT, 32))):
                            rot = qkp.tile([D, T], F32R, tag="rot")
                            slabs = []
                            for wh in range(2):
                                ws = wqkp.tile([128, CT // 2, 128], F32R, tag="wqk")
                                nc.sync.dma_start(
                                    ws[:],
                                    wT.rearrange("(ct p) o -> p ct o", p=128)[
                                        :, wh * (CT // 2):(wh + 1) * (CT // 2),
                                        h * D:(h + 1) * D])
                                slabs.append(ws)
                            for ci in range(NCH):
                                ps = ps_acc.tile([128, CH], F32, tag="acc")
                                for ct in range(CT):
                                    nc.tensor.matmul(ps[:],
                                                     slabs[ct // (CT // 2)][:, ct % (CT // 2), :],
                                                     x_sb[:, ct, ci * CH:(ci + 1) * CH],
                                                     start=(ct == 0), stop=False)
                                nc.tensor.matmul(ps[:], dB_sb[urow:urow + R, h * D:(h + 1) * D],
                                                 u_sb[urow:urow + R, ci * CH:(ci + 1) * CH],
                                                 start=False, stop=True)
                                # RoPE: PSUM -> SBUF copy, shift, mul, add -> rot (f32r)
                                q0 = rope.tile([D, CH], F32, tag="q0")
                                nc.vector.tensor_copy(q0[:], ps[:])
                                sh = rope.tile([D, CH], F32, tag="sh")
                                nc.sync.dma_start(sh[0:64, :], q0[64:128, :])
                                nc.sync.dma_start(sh[64:128, :], q0[0:64, :])
                                nc.vector.tensor_mul(sh[:], sh[:],
                                                     sin_sb[:, ci * CH:(ci + 1) * CH])
                                nc.vector.tensor_mul(q0[:], q0[:],
                                                     cos_sb[:, ci * CH:(ci + 1) * CH])
                                nc.vector.tensor_add(rot[:, ci * CH:(ci + 1) * CH],
                                                     q0[:], sh[:])
                            rots.append(rot)
                        qr, kr = rots

                        _mark(nc, f'a1_{h}')
                        # A1 (2-bank psum per s_tile) + single exp [128, T]
                        pts = []
                        for part in range(IT // PTP):
                            pT = ptp.tile([128, PTP, T], F32R, tag="pT")
                            for sp in range(PTP):
                                st = part * PTP + sp
                                for ci in range(NCH):
                                    ps = ps_s.tile([128, CH], F32, tag="s")
                                    nc.tensor.matmul(ps[:],
                                                     kr[:, st * 128:(st + 1) * 128],
                                                     qr[:, ci * CH:(ci + 1) * CH],
                                                     start=True, stop=True)
                                    nc.scalar.activation(pT[:, sp, ci * CH:(ci + 1) * CH],
                                                         ps[:], AF.Exp, scale=SCALE)
                            pts.append(pT)

                        _mark(nc, f'l_{h}')
                        # l = column sums of p^T ; rb = broadcast reciprocal
                        rb = att.tile([128, T], F32, tag="rb")
                        l_sb = tabs.tile([1, T], F32R, tag="l_sb")
                        for ci in range(NCH):
                            l_ps = ps_lb.tile([1, CH], F32, tag="lb")
                            for st in range(IT):
                                nc.tensor.matmul(l_ps[:], ones128,
                                                 pts[st // PTP][:, st % PTP,
                                                                ci * CH:(ci + 1) * CH],
                                                 start=(st == 0), stop=(st == IT - 1))
                            nc.scalar.activation(l_sb[:, ci * CH:(ci + 1) * CH], l_ps[:],
                                                 AF.Copy)
                            lb = ps_lb.tile([128, CH], F32, tag="lb")
                            nc.tensor.matmul(lb[:], ones1, l_sb[:, ci * CH:(ci + 1) * CH],
                                             start=True, stop=True)
                            nc.vector.reciprocal_approx_fast(
                                out=rb[:, ci * CH:(ci + 1) * CH], in_=lb[:])

                        _mark(nc, f'a2_{h}')
                        # A2 + scale + spill + per-head uo accumulation
                        y_sc = att.tile([D, T], F32R, tag="y_sc")
                        for ci in range(NCH):
                            yp = ps_y.tile([D, CH], F32, tag="y")
                            for st in range(IT):
                                nc.tensor.matmul(yp[:], v_sb[:, st, h * D:(h + 1) * D],
                                                 pts[st // PTP][:, st % PTP,
                                                                ci * CH:(ci + 1) * CH],
                                                 start=(st == 0), stop=(st == IT - 1))
                            nc.vector.tensor_mul(y_sc[:, ci * CH:(ci + 1) * CH], yp[:],
                                                 rb[:, ci * CH:(ci + 1) * CH])
                        nc.sync.dma_start(y_spill[h], y_sc[:])
                        for ci in range(NCH):
                            ps_uoh = ps_lb.tile([R, CH], F32, tag="lb")
                            nc.tensor.matmul(ps_uoh[:], doAT_sb[:, h, :],
                                             y_sc[:, ci * CH:(ci + 1) * CH],
                                             start=True, stop=True)
                            if h == 0:
                                nc.vector.tensor_copy(uo_acc[:, ci * CH:(ci + 1) * CH],
                                                      ps_uoh[:])
                            else:
                                nc.vector.tensor_add(uo_acc[:, ci * CH:(ci + 1) * CH],
                                                     uo_acc[:, ci * CH:(ci + 1) * CH],
                                                     ps_uoh[:])

            # ============ phase 2: out^T = Wo-half contraction + LoRA ============
            _mark(nc, 'p2')
            with tc.tile_pool(name="p2", bufs=1) as p2p, \
                 tc.tile_pool(name="wo", bufs=3) as wop, \
                 tc.tile_pool(name="outp", bufs=3) as outp:
                y_sb = p2p.tile([128, HH, T], F32R)
                for hh in range(HH):
                    nc.sync.dma_start(y_sb[:, hh, :], y_spill[hh])
                doB_sb = p2p.tile([R, C], F32R)
                nc.sync.dma_start(doB_sb[:], doB[:])
                uo_sb = p2p.tile([R, T], F32R)
                nc.vector.tensor_copy(uo_sb[:], uo_acc[:])

                wor = WoT.rearrange("(it p) o -> p it o", p=128)
                for ot in range(C // 128):
                    wo = wop.tile([128, IT, 128], F32R, tag="wo")
                    nc.sync.dma_start(wo[:, 0:4, :], wor[:, 0:4, ot * 128:(ot + 1) * 128])
                    nc.sync.dma_start(wo[:, 4:8, :], wor[:, 4:8, ot * 128:(ot + 1) * 128])
                    for ci in range(NCH):
                        ps = ps_acc.tile([128, CH], F32, tag="acc")
                        for it in range(IT):
                            nc.tensor.matmul(ps[:], wo[:, it, :],
                                             y_sb[:, it, ci * CH:(ci + 1) * CH],
                                             start=(it == 0), stop=False)
                        nc.tensor.matmul(ps[:], doB_sb[:, ot * 128:(ot + 1) * 128],
                                         uo_sb[:, ci * CH:(ci + 1) * CH],
                                         start=False, stop=True)
                        o_sb = outp.tile([128, CH], F32, tag="o")
                        nc.scalar.activation(o_sb[:], ps[:], AF.Copy)
                        nc.sync.dma_start(outT[ot * 128:(ot + 1) * 128,
                                               ci * CH:(ci + 1) * CH], o_sb[:])

    nc.compile()
    return nc


def _rope_tables():
    inv = (1.0 / (ROPE_BASE ** (np.arange(0, D, 2, dtype=np.float32) / np.float32(D)))).astype(np.float32)
    t_ar = np.arange(T, dtype=np.float32)
    fr = t_ar[:, None] * inv[None, :]
    emb = np.concatenate([fr, fr], axis=1)          # [T, D]
    cos = np.cos(emb).astype(np.float32).T.copy()   # [D, T]
    sin = np.sin(emb).astype(np.float32).T.copy()
    sins = sin.copy()
    sins[:64, :] *= -1.0
    return np.ascontiguousarray(cos), np.ascontiguousarray(sins)


def kernel(x, qkvo_delta, Wq, Wk, Wv, Wo):
    global last_result
    x = np.asarray(x, dtype=np.float32)
    qkvo_delta = np.asarray(qkvo_delta, dtype=np.float32)
    Wq = np.asarray(Wq, dtype=np.float32)
    Wk = np.asarray(Wk, dtype=np.float32)
    Wv = np.asarray(Wv, dtype=np.float32)
    Wo = np.asarray(Wo, dtype=np.float32)

    if "nc" not in _compiled:
        _compiled["nc"] = _build_nc()
    nc = _compiled["nc"]

    cos, sins = _rope_tables()
    d = qkvo_delta.reshape(B, 8, R, C)
    dqA, dqB, dkA, dkB, dvA, dvB, doA, doB = (d[:, i] for i in range(8))

    in_maps = []
    for core in range(8):
        b, half = core // 2, core % 2
        sl = slice(half * HALF, (half + 1) * HALF)
        dAT = np.zeros((C, 80), dtype=np.float32)
        dAT[:, 0:16] = dqA[b].T
        dAT[:, 32:48] = dkA[b].T
        dAT[:, 64:80] = dvA[b].T
        dBp = np.zeros((80, HALF), dtype=np.float32)
        dBp[0:16] = dqB[b][:, sl]
        dBp[32:48] = dkB[b][:, sl]
        dBp[64:80] = dvB[b][:, sl]
        in_maps.append({
            "xT": np.ascontiguousarray(x[b].T),
            "WqT": np.ascontiguousarray(Wq[sl, :].T),
            "WkT": np.ascontiguousarray(Wk[sl, :].T),
            "WvT": np.ascontiguousarray(Wv[sl, :].T),
            "WoT": np.ascontiguousarray(Wo[:, sl].T),
            "dAT": dAT,
            "dBp": dBp,
            "doAT": np.ascontiguousarray(doA[b][:, sl].T),
            "doB": np.ascontiguousarray(doB[b]),
            "cosT": cos,
            "sinTs": sins,
        })

    trace = bool(int(os.environ.get("KERNEL_TRACE", "0")))
    res = bass_utils.run_bass_kernel_spmd(
        nc, in_maps, core_ids=list(range(8)), trace=trace)
    last_result = res

    out = np.empty((B, T, C), dtype=np.float32)
    for b in range(B):
        acc = res.results[2 * b]["outT"].astype(np.float32) + \
            res.results[2 * b + 1]["outT"].astype(np.float32)
        out[b] = acc.T
    return out



# revision 6
# speedup vs baseline: 1.2646x; 1.2646x over previous
"""Trainium2 Bass kernel for per-sample-LoRA causal self-attention (non-causal SDPA).

Sharding: 8 cores = (batch b in 0..3) x (channel-half in 0..1).
Host merges the per-sample LoRA deltas into the weights (the sharding hint's
"each device gets its own ... merged weights"), so the device runs a plain
dense attention block: each core computes q/k/v for its 1024 output channels
(8 heads) of sample b, runs attention for those heads, and produces a partial
output projection (contraction over its half of the y channels). Host sums
the two partials per sample and transposes back.

All matmuls run in bf16 (PE full rate, FWL weight loads, half the DMA/SBUF
of f32r); accumulation is f32 in PSUM.
"""

import os
import sys

sys.path.insert(0, "/opt/trn_rl_repo")

import numpy as np
import ml_dtypes

import concourse.bass as bass  # noqa: F401
import concourse.mybir as mybir
import concourse.tile as tile
from concourse import bacc, bass_utils

F32 = mybir.dt.float32
BF = mybir.dt.bfloat16
AF = mybir.ActivationFunctionType

B, T, C = 4, 1024, 2048
H, D, R = 16, 128, 16
HALF = C // 2          # output channels per core
HH = HALF // D         # heads per core = 8
CT = C // 128          # contraction tiles over C = 16
IT = HALF // 128       # contraction tiles over half = 8
CH = 512               # t/free chunk
NCH = T // CH          # = 2
N_WARM = 24            # PE warm-up matmuls while x DMA lands
SCALE = 1.0 / float(np.sqrt(D))
ROPE_BASE = 10000.0

_compiled = {}
last_result = None     # BassKernelResults of the most recent run (for test harness)
PHASES = []            # (label, first instruction number) build-time markers


def _mark(nc, label):
    PHASES.append((label, int(nc.get_next_instruction_name().split("-")[1])))


def _build_nc():
    nc = bacc.Bacc("TRN2", target_bir_lowering=False, debug=False, num_devices=8)

    xT = nc.dram_tensor("xT", [C, T], BF, kind="ExternalInput").ap()
    WqT = nc.dram_tensor("WqT", [C, HALF], BF, kind="ExternalInput").ap()
    WkT = nc.dram_tensor("WkT", [C, HALF], BF, kind="ExternalInput").ap()
    WvT = nc.dram_tensor("WvT", [C, HALF], BF, kind="ExternalInput").ap()
    WoT = nc.dram_tensor("WoT", [HALF, C], BF, kind="ExternalInput").ap()
    cosT = nc.dram_tensor("cosT", [D, T], BF, kind="ExternalInput").ap()
    sinTs = nc.dram_tensor("sinTs", [D, T], BF, kind="ExternalInput").ap()
    outT = nc.dram_tensor("outT", [C, T], F32, kind="ExternalOutput").ap()

    with tile.TileContext(nc) as tc:
        with tc.tile_pool(name="tabs", bufs=1) as tabs, \
             tc.tile_pool(name="ps_acc", bufs=2, space="PSUM") as ps_acc, \
             tc.tile_pool(name="ps_s", bufs=2, space="PSUM") as ps_s, \
             tc.tile_pool(name="ps_y", bufs=2, space="PSUM") as ps_y, \
             tc.tile_pool(name="ps_l", bufs=1, space="PSUM") as ps_l, \
             tc.tile_pool(name="ps_lb", bufs=1, space="PSUM") as ps_lb:

            _mark(nc, 'init')
            # ---------------- resident tables ----------------
            x_sb = tabs.tile([128, CT, T], BF)
            v_sb = tabs.tile([128, IT, HALF], BF)     # [t_in_tile, t_tile, vo]
            y_sb = tabs.tile([128, IT, T], BF)        # [d, head, t] resident
            cos_sb = tabs.tile([D, T], BF)
            sin_sb = tabs.tile([D, T], BF)
            const_f = tabs.tile([128, 130], F32)
            const_b = tabs.tile([128, 130], BF)
            ones128 = const_b[:, 0:1]
            ones1 = const_b[0:1, 1:129]

            # warm-up: PE busy from t~0 so HAM is unthrottled when x lands
            nc.gpsimd.memset(const_f[:], 1.0)
            nc.vector.tensor_copy(const_b[:], const_f[:])
            ps_w = ps_acc.tile([128, CH], F32, tag="acc")
            for _ in range(N_WARM):
                nc.tensor.matmul(ps_w[:, 0:128], const_b[:, 1:129],
                                 const_b[:, 1:129], start=True, stop=True)

            nc.sync.dma_start(cos_sb[:], cosT[:])
            nc.sync.dma_start(sin_sb[:], sinTs[:])
            xr = xT.rearrange("(ct p) t -> p ct t", p=128)
            for ct in range(CT):
                nc.sync.dma_start(x_sb[:, ct, :], xr[:, ct, :])

            # ---------------- v projection ----------------
            _mark(nc, 'v')
            with tc.tile_pool(name="wv", bufs=2) as wvp:
                wvr = WvT.rearrange("(ct p) o -> p ct o", p=128)
                for ci in range(2):                   # vo chunk of 512
                    wv = wvp.tile([128, CT, CH], BF, tag="wv")
                    for wg in range(4):
                        nc.sync.dma_start(
                            wv[:, 4 * wg:4 * wg + 4, :],
                            wvr[:, 4 * wg:4 * wg + 4, ci * CH:(ci + 1) * CH])
                    for tt in range(IT):
                        ps = ps_acc.tile([128, CH], F32, tag="acc")
                        for ct in range(CT):
                            nc.tensor.matmul(ps[:],
                                             x_sb[:, ct, tt * 128:(tt + 1) * 128],
                                             wv[:, ct, :],
                                             start=(ct == 0), stop=(ct == CT - 1))
                        nc.scalar.activation(v_sb[:, tt, ci * CH:(ci + 1) * CH],
                                             ps[:], AF.Copy)

            # ---------------- per-head: qk proj + RoPE + attention ----------------
            with tc.tile_pool(name="wqk", bufs=4) as wqkp, \
                 tc.tile_pool(name="shp", bufs=4) as shp, \
                 tc.tile_pool(name="qk", bufs=4) as qkp, \
                 tc.tile_pool(name="ptp", bufs=2) as ptp, \
                 tc.tile_pool(name="att", bufs=2) as att:
                for h in range(HH):
                    _mark(nc, f'qk{h}')
                    rots = []
                    for wT in (WqT, WkT):
                        rot = qkp.tile([D, T], BF, tag="rot")
                        ws = wqkp.tile([128, CT, D], BF, tag="wqk")
                        wr = wT.rearrange("(ct p) o -> p ct o", p=128)
                        nc.sync.dma_start(ws[:, 0:8, :], wr[:, 0:8, h * D:(h + 1) * D])
                        nc.sync.dma_start(ws[:, 8:16, :], wr[:, 8:16, h * D:(h + 1) * D])
                        for ci in range(NCH):
                            ps = ps_acc.tile([128, CH], F32, tag="acc")
                            for ct in range(CT):
                                nc.tensor.matmul(ps[:], ws[:, ct, :],
                                                 x_sb[:, ct, ci * CH:(ci + 1) * CH],
                                                 start=(ct == 0), stop=(ct == CT - 1))
                            # RoPE: rot = ps*cos + shift(ps)*sin (sin sign
                            # pre-folded). Shift runs via bf16 SBUF->SBUF DMA
                            # of an ACT-copied q0; DVE ops stay mostly 16-bit.
                            q0 = shp.tile([D, CH], BF, tag="q0")
                            nc.scalar.activation(q0[:], ps[:], AF.Copy)
                            sh = shp.tile([D, CH], BF, tag="sh")
                            nc.sync.dma_start(sh[0:64, :], q0[64:128, :])
                            nc.sync.dma_start(sh[64:128, :], q0[0:64, :])
                            nc.vector.tensor_mul(rot[:, ci * CH:(ci + 1) * CH], ps[:],
                                                 cos_sb[:, ci * CH:(ci + 1) * CH])
                            nc.vector.tensor_mul(sh[:], sh[:],
                                                 sin_sb[:, ci * CH:(ci + 1) * CH])
                            nc.vector.tensor_add(rot[:, ci * CH:(ci + 1) * CH],
                                                 rot[:, ci * CH:(ci + 1) * CH], sh[:])
                        rots.append(rot)
                    qr, kr = rots

                    _mark(nc, f'a1_{h}')
                    pT = ptp.tile([128, IT, T], BF, tag="pT")
                    for st in range(IT):
                        for ci in range(NCH):
                            ps = ps_s.tile([128, CH], F32, tag="s")
                            nc.tensor.matmul(ps[:], kr[:, st * 128:(st + 1) * 128],
                                             qr[:, ci * CH:(ci + 1) * CH],
                                             start=True, stop=True)
                            nc.scalar.activation(pT[:, st, ci * CH:(ci + 1) * CH],
                                                 ps[:], AF.Exp, scale=SCALE)

                    _mark(nc, f'l_{h}')
                    # softmax denominators: column sums of p^T, broadcast reciprocal
                    rb = att.tile([128, T], F32, tag="rb")
                    l_sb = att.tile([1, T], BF, tag="l_sb")
                    for ci in range(NCH):
                        l_ps = ps_l.tile([1, CH], F32, tag="l")
                        for st in range(IT):
                            nc.tensor.matmul(l_ps[:], ones128,
                                             pT[:, st, ci * CH:(ci + 1) * CH],
                                             start=(st == 0), stop=(st == IT - 1))
                        nc.scalar.activation(l_sb[:, ci * CH:(ci + 1) * CH], l_ps[:],
                                             AF.Copy)
                        lb = ps_lb.tile([128, CH], F32, tag="lb")
                        nc.tensor.matmul(lb[:], ones1, l_sb[:, ci * CH:(ci + 1) * CH],
                                         start=True, stop=True)
                        nc.vector.reciprocal_approx_fast(
                            out=rb[:, ci * CH:(ci + 1) * CH], in_=lb[:])

                    _mark(nc, f'a2_{h}')
                    for ci in range(NCH):
                        yp = ps_y.tile([D, CH], F32, tag="y")
                        for st in range(IT):
                            nc.tensor.matmul(yp[:], v_sb[:, st, h * D:(h + 1) * D],
                                             pT[:, st, ci * CH:(ci + 1) * CH],
                                             start=(st == 0), stop=(st == IT - 1))
                        nc.vector.tensor_mul(y_sb[:, h, ci * CH:(ci + 1) * CH], yp[:],
                                             rb[:, ci * CH:(ci + 1) * CH])

            # ---------------- output projection (partial over this half) ----------
            _mark(nc, 'p2')
            with tc.tile_pool(name="wo", bufs=3) as wop, \
                 tc.tile_pool(name="outp", bufs=3) as outp:
                wor = WoT.rearrange("(it p) o -> p it o", p=128)
                for ot in range(C // 128):
                    wo = wop.tile([128, IT, 128], BF, tag="wo")
                    nc.sync.dma_start(wo[:], wor[:, :, ot * 128:(ot + 1) * 128])
                    for ci in range(NCH):
                        ps = ps_acc.tile([128, CH], F32, tag="acc")
                        for it in range(IT):
                            nc.tensor.matmul(ps[:], wo[:, it, :],
                                             y_sb[:, it, ci * CH:(ci + 1) * CH],
                                             start=(it == 0), stop=(it == IT - 1))
                        o_sb = outp.tile([128, CH], F32, tag="o")
                        nc.scalar.activation(o_sb[:], ps[:], AF.Copy)
                        nc.sync.dma_start(outT[ot * 128:(ot + 1) * 128,
                                               ci * CH:(ci + 1) * CH], o_sb[:])

    nc.compile()
    return nc


def _rope_tables():
    inv = (1.0 / (ROPE_BASE ** (np.arange(0, D, 2, dtype=np.float32) / np.float32(D)))).astype(np.float32)
    t_ar = np.arange(T, dtype=np.float32)
    fr = t_ar[:, None] * inv[None, :]
    emb = np.concatenate([fr, fr], axis=1)          # [T, D]
    cos = np.cos(emb).astype(np.float32).T.copy()   # [D, T]
    sin = np.sin(emb).astype(np.float32).T.copy()
    sins = sin.copy()
    sins[:64, :] *= -1.0                            # rotate-half sign folded in
    return _bf(cos), _bf(sins)


def _bf(a):
    return np.ascontiguousarray(a).astype(ml_dtypes.bfloat16)


def kernel(x, qkvo_delta, Wq, Wk, Wv, Wo):
    global last_result
    x = np.asarray(x, dtype=np.float32)
    qkvo_delta = np.asarray(qkvo_delta, dtype=np.float32)
    Wq = np.asarray(Wq, dtype=np.float32)
    Wk = np.asarray(Wk, dtype=np.float32)
    Wv = np.asarray(Wv, dtype=np.float32)
    Wo = np.asarray(Wo, dtype=np.float32)

    if "nc" not in _compiled:
        _compiled["nc"] = _build_nc()
    nc = _compiled["nc"]

    cos, sins = _rope_tables()
    d = qkvo_delta.reshape(B, 8, R, C)

    in_maps = []
    for b in range(B):
        # merge per-sample LoRA deltas into the weights (W + dB^T @ dA)
        Wqm = Wq + d[b, 1].T @ d[b, 0]
        Wkm = Wk + d[b, 3].T @ d[b, 2]
        Wvm = Wv + d[b, 5].T @ d[b, 4]
        Wom = Wo + d[b, 7].T @ d[b, 6]
        xTb = _bf(x[b].T)
        for half in range(2):
            sl = slice(half * HALF, (half + 1) * HALF)
            in_maps.append({
                "xT": xTb,
                "WqT": _bf(Wqm[sl, :].T),
                "WkT": _bf(Wkm[sl, :].T),
                "WvT": _bf(Wvm[sl, :].T),
                "WoT": _bf(Wom[:, sl].T),
                "cosT": cos,
                "sinTs": sins,
            })

    trace = bool(int(os.environ.get("KERNEL_TRACE", "0")))
    res = bass_utils.run_bass_kernel_spmd(
        nc, in_maps, core_ids=list(range(8)), trace=trace)
    last_result = res

    out = np.empty((B, T, C), dtype=np.float32)
    for b in range(B):
        acc = res.results[2 * b]["outT"].astype(np.float32) + \
            res.results[2 * b + 1]["outT"].astype(np.float32)
        out[b] = acc.T
    return out


# revision 7
# speedup vs baseline: 1.3104x; 1.0362x over previous
"""Trainium2 Bass kernel for per-sample-LoRA causal self-attention (non-causal SDPA).

Sharding: 8 cores = (batch b in 0..3) x (channel-half in 0..1).
Host merges the per-sample LoRA deltas into the weights (the sharding hint's
"each device gets its own ... merged weights"), so the device runs a plain
dense attention block: each core computes q/k/v for its 1024 output channels
(8 heads) of sample b, runs attention for those heads, and produces a partial
output projection (contraction over its half of the y channels). Host sums
the two partials per sample and transposes back.

All matmuls run in bf16 (PE full rate, FWL weight loads, half the DMA/SBUF
of f32r); accumulation is f32 in PSUM. Weights are pre-tiled on the host into
SBUF layout so every DMA moves 2-4KB contiguous lines per partition.
"""

import os
import sys

sys.path.insert(0, "/opt/trn_rl_repo")

import numpy as np
import ml_dtypes

import concourse.bass as bass  # noqa: F401
import concourse.mybir as mybir
import concourse.tile as tile
from concourse import bacc, bass_utils

F32 = mybir.dt.float32
BF = mybir.dt.bfloat16
AF = mybir.ActivationFunctionType

B, T, C = 4, 1024, 2048
H, D, R = 16, 128, 16
HALF = C // 2          # output channels per core
HH = HALF // D         # heads per core = 8
CT = C // 128          # contraction tiles over C = 16
IT = HALF // 128       # contraction tiles over half = 8
CH = 512               # t/free chunk
NCH = T // CH          # = 2
N_WARM = 16            # PE warm-up matmuls while x DMA lands
SCALE = 1.0 / float(np.sqrt(D))
ROPE_BASE = 10000.0

_compiled = {}
last_result = None     # BassKernelResults of the most recent run (for test harness)
PHASES = []            # (label, first instruction number) build-time markers


def _mark(nc, label):
    PHASES.append((label, int(nc.get_next_instruction_name().split("-")[1])))


def _build_nc():
    nc = bacc.Bacc("TRN2", target_bir_lowering=False, debug=False, num_devices=8)

    xT = nc.dram_tensor("xT", [C, T], BF, kind="ExternalInput").ap()
    # pre-tiled weights: per-partition lines are contiguous SBUF tile rows
    Wqk_t = nc.dram_tensor("Wqk_t", [2 * HH * 128, CT * D], BF,
                           kind="ExternalInput").ap()
    Wv_t = nc.dram_tensor("Wv_t", [2 * 128, CT * CH], BF, kind="ExternalInput").ap()
    Wo_t = nc.dram_tensor("Wo_t", [CT * 128, IT * 128], BF,
                          kind="ExternalInput").ap()
    cosT = nc.dram_tensor("cosT", [D, T], BF, kind="ExternalInput").ap()
    sinTs = nc.dram_tensor("sinTs", [D, T], BF, kind="ExternalInput").ap()
    outT = nc.dram_tensor("outT", [C, T], F32, kind="ExternalOutput").ap()

    wqk_r = Wqk_t.rearrange("(s p) e -> s p e", p=128)   # s = proj*8 + head
    wv_r = Wv_t.rearrange("(s p) e -> s p e", p=128)     # s = ci
    wo_r = Wo_t.rearrange("(s p) e -> s p e", p=128)     # s = ot

    with tile.TileContext(nc) as tc:
        with tc.tile_pool(name="tabs", bufs=1) as tabs, \
             tc.tile_pool(name="ps_acc", bufs=2, space="PSUM") as ps_acc, \
             tc.tile_pool(name="ps_s", bufs=2, space="PSUM") as ps_s, \
             tc.tile_pool(name="ps_y", bufs=2, space="PSUM") as ps_y, \
             tc.tile_pool(name="ps_l", bufs=1, space="PSUM") as ps_l, \
             tc.tile_pool(name="ps_lb", bufs=1, space="PSUM") as ps_lb:

            _mark(nc, 'init')
            # ---------------- resident tables ----------------
            x_sb = tabs.tile([128, CT, T], BF)
            v_sb = tabs.tile([128, IT, HALF], BF)     # [t_in_tile, t_tile, vo]
            y_sb = tabs.tile([128, IT, T], BF)        # [d, head, t] resident
            cos_sb = tabs.tile([D, T], BF)
            sin_sb = tabs.tile([D, T], BF)
            const_f = tabs.tile([128, 130], F32)
            const_b = tabs.tile([128, 130], BF)
            ones128 = const_b[:, 0:1]
            ones1 = const_b[0:1, 1:129]

            # warm-up: PE busy from t~0 so HAM is unthrottled when x lands
            nc.gpsimd.memset(const_f[:], 1.0)
            nc.vector.tensor_copy(const_b[:], const_f[:])
            ps_w = ps_acc.tile([128, CH], F32, tag="acc")
            for _ in range(N_WARM):
                nc.tensor.matmul(ps_w[:, 0:128], const_b[:, 1:129],
                                 const_b[:, 1:129], start=True, stop=True)

            # ---------------- v projection ----------------
            _mark(nc, 'v')
            with tc.tile_pool(name="wv", bufs=2) as wvp, \
                 tc.tile_pool(name="wqk", bufs=4) as wqkp:
                # DMA order: first wv chunk, then x, tables, head-0 slabs
                wv0 = wvp.tile([128, CT, CH], BF, tag="wv")
                for g in range(4):
                    nc.sync.dma_start(wv0[:, 4 * g:4 * g + 4, :],
                                      wv_r[0, :, g * 4 * CH:(g + 1) * 4 * CH])
                xr = xT.rearrange("(ct p) t -> p ct t", p=128)
                for ct in range(CT):
                    nc.sync.dma_start(x_sb[:, ct, :], xr[:, ct, :])
                nc.sync.dma_start(cos_sb[:], cosT[:])
                nc.sync.dma_start(sin_sb[:], sinTs[:])
                ws0 = []
                for pi in range(2):
                    ws = wqkp.tile([128, CT, D], BF, tag="wqk")
                    nc.sync.dma_start(ws[:], wqk_r[pi * HH, :, :])
                    ws0.append(ws)
                wv1 = wvp.tile([128, CT, CH], BF, tag="wv")
                for g in range(4):
                    nc.sync.dma_start(wv1[:, 4 * g:4 * g + 4, :],
                                      wv_r[1, :, g * 4 * CH:(g + 1) * 4 * CH])

                for ci, wv in ((0, wv0), (1, wv1)):
                    for tt in range(IT):
                        ps = ps_acc.tile([128, CH], F32, tag="acc")
                        for ct in range(CT):
                            nc.tensor.matmul(ps[:],
                                             x_sb[:, ct, tt * 128:(tt + 1) * 128],
                                             wv[:, ct, :],
                                             start=(ct == 0), stop=(ct == CT - 1))
                        nc.scalar.activation(v_sb[:, tt, ci * CH:(ci + 1) * CH],
                                             ps[:], AF.Copy)

                # ---------------- per-head: qk proj + RoPE + attention --------
                with tc.tile_pool(name="shp", bufs=4) as shp, \
                     tc.tile_pool(name="qk", bufs=4) as qkp, \
                     tc.tile_pool(name="ptp", bufs=2) as ptp, \
                     tc.tile_pool(name="att", bufs=2) as att:
                    for h in range(HH):
                        _mark(nc, f'qk{h}')
                        rots = []
                        for pi in range(2):
                            rot = qkp.tile([D, T], BF, tag="rot")
                            if h == 0:
                                ws = ws0[pi]
                            else:
                                ws = wqkp.tile([128, CT, D], BF, tag="wqk")
                                nc.sync.dma_start(ws[:], wqk_r[pi * HH + h, :, :])
                            for ci in range(NCH):
                                ps = ps_acc.tile([128, CH], F32, tag="acc")
                                for ct in range(CT):
                                    nc.tensor.matmul(ps[:], ws[:, ct, :],
                                                     x_sb[:, ct, ci * CH:(ci + 1) * CH],
                                                     start=(ct == 0),
                                                     stop=(ct == CT - 1))
                                # RoPE: rot = ps*cos + shift(ps)*sin (sin sign
                                # pre-folded); shift via bf16 SBUF->SBUF DMA of
                                # an ACT-copied q0.
                                q0 = shp.tile([D, CH], BF, tag="q0")
                                nc.scalar.activation(q0[:], ps[:], AF.Copy)
                                sh = shp.tile([D, CH], BF, tag="sh")
                                nc.sync.dma_start(sh[0:64, :], q0[64:128, :])
                                nc.sync.dma_start(sh[64:128, :], q0[0:64, :])
                                nc.vector.tensor_mul(rot[:, ci * CH:(ci + 1) * CH],
                                                     ps[:],
                                                     cos_sb[:, ci * CH:(ci + 1) * CH])
                                nc.vector.tensor_mul(sh[:], sh[:],
                                                     sin_sb[:, ci * CH:(ci + 1) * CH])
                                nc.vector.tensor_add(rot[:, ci * CH:(ci + 1) * CH],
                                                     rot[:, ci * CH:(ci + 1) * CH],
                                                     sh[:])
                            rots.append(rot)
                        qr, kr = rots

                        _mark(nc, f'a1_{h}')
                        pT = ptp.tile([128, IT, T], BF, tag="pT")
                        for st in range(IT):
                            for ci in range(NCH):
                                ps = ps_s.tile([128, CH], F32, tag="s")
                                nc.tensor.matmul(ps[:], kr[:, st * 128:(st + 1) * 128],
                                                 qr[:, ci * CH:(ci + 1) * CH],
                                                 start=True, stop=True)
                                nc.scalar.activation(pT[:, st, ci * CH:(ci + 1) * CH],
                                                     ps[:], AF.Exp, scale=SCALE)

                        _mark(nc, f'l_{h}')
                        # softmax denominators: column sums of p^T + broadcast
                        rb = att.tile([128, T], F32, tag="rb")
                        l_sb = att.tile([1, T], BF, tag="l_sb")
                        for ci in range(NCH):
                            l_ps = ps_l.tile([1, CH], F32, tag="l")
                            for st in range(IT):
                                nc.tensor.matmul(l_ps[:], ones128,
                                                 pT[:, st, ci * CH:(ci + 1) * CH],
                                                 start=(st == 0), stop=(st == IT - 1))
                            nc.scalar.activation(l_sb[:, ci * CH:(ci + 1) * CH],
                                                 l_ps[:], AF.Copy)
                            lb = ps_lb.tile([128, CH], F32, tag="lb")
                            nc.tensor.matmul(lb[:], ones1,
                                             l_sb[:, ci * CH:(ci + 1) * CH],
                                             start=True, stop=True)
                            nc.vector.reciprocal_approx_fast(
                                out=rb[:, ci * CH:(ci + 1) * CH], in_=lb[:])

                        _mark(nc, f'a2_{h}')
                        for ci in range(NCH):
                            yp = ps_y.tile([D, CH], F32, tag="y")
                            for st in range(IT):
                                nc.tensor.matmul(yp[:], v_sb[:, st, h * D:(h + 1) * D],
                                                 pT[:, st, ci * CH:(ci + 1) * CH],
                                                 start=(st == 0), stop=(st == IT - 1))
                            nc.vector.tensor_mul(y_sb[:, h, ci * CH:(ci + 1) * CH],
                                                 yp[:], rb[:, ci * CH:(ci + 1) * CH])

            # ---------------- output projection (partial over this half) ------
            _mark(nc, 'p2')
            with tc.tile_pool(name="wo", bufs=3) as wop, \
                 tc.tile_pool(name="outp", bufs=3) as outp:
                for ot in range(C // 128):
                    wo = wop.tile([128, IT, 128], BF, tag="wo")
                    nc.sync.dma_start(wo[:], wo_r[ot, :, :])
                    for ci in range(NCH):
                        ps = ps_acc.tile([128, CH], F32, tag="acc")
                        for it in range(IT):
                            nc.tensor.matmul(ps[:], wo[:, it, :],
                                             y_sb[:, it, ci * CH:(ci + 1) * CH],
                                             start=(it == 0), stop=(it == IT - 1))
                        o_sb = outp.tile([128, CH], F32, tag="o")
                        nc.scalar.activation(o_sb[:], ps[:], AF.Copy)
                        nc.sync.dma_start(outT[ot * 128:(ot + 1) * 128,
                                               ci * CH:(ci + 1) * CH], o_sb[:])

    nc.compile()
    return nc


def _rope_tables():
    inv = (1.0 / (ROPE_BASE ** (np.arange(0, D, 2, dtype=np.float32) / np.float32(D)))).astype(np.float32)
    t_ar = np.arange(T, dtype=np.float32)
    fr = t_ar[:, None] * inv[None, :]
    emb = np.concatenate([fr, fr], axis=1)          # [T, D]
    cos = np.cos(emb).astype(np.float32).T.copy()   # [D, T]
    sin = np.sin(emb).astype(np.float32).T.copy()
    sins = sin.copy()
    sins[:64, :] *= -1.0                            # rotate-half sign folded in
    return _bf(cos), _bf(sins)


def _bf(a):
    return np.ascontiguousarray(a).astype(ml_dtypes.bfloat16)


def kernel(x, qkvo_delta, Wq, Wk, Wv, Wo):
    global last_result
    x = np.asarray(x, dtype=np.float32)
    qkvo_delta = np.asarray(qkvo_delta, dtype=np.float32)
    Wq = np.asarray(Wq, dtype=np.float32)
    Wk = np.asarray(Wk, dtype=np.float32)
    Wv = np.asarray(Wv, dtype=np.float32)
    Wo = np.asarray(Wo, dtype=np.float32)

    if "nc" not in _compiled:
        _compiled["nc"] = _build_nc()
    nc = _compiled["nc"]

    cos, sins = _rope_tables()
    d = qkvo_delta.reshape(B, 8, R, C)

    in_maps = []
    for b in range(B):
        # merge per-sample LoRA deltas into the weights (W + dB^T @ dA)
        Wqm = Wq + d[b, 1].T @ d[b, 0]
        Wkm = Wk + d[b, 3].T @ d[b, 2]
        Wvm = Wv + d[b, 5].T @ d[b, 4]
        Wom = Wo + d[b, 7].T @ d[b, 6]
        xTb = _bf(x[b].T)
        for half in range(2):
            sl = slice(half * HALF, (half + 1) * HALF)
            # tile [in_tiles(16) x 128p, head-or-chunk, cols] into SBUF order
            wq = Wqm[sl, :].T.reshape(CT, 128, HH, D).transpose(2, 1, 0, 3)
            wk = Wkm[sl, :].T.reshape(CT, 128, HH, D).transpose(2, 1, 0, 3)
            wv = Wvm[sl, :].T.reshape(CT, 128, 2, CH).transpose(2, 1, 0, 3)
            wo = Wom[:, sl].T.reshape(IT, 128, CT, 128).transpose(2, 1, 0, 3)
            in_maps.append({
                "xT": xTb,
                "Wqk_t": _bf(np.concatenate([wq, wk]).reshape(2 * HH * 128, CT * D)),
                "Wv_t": _bf(wv.reshape(2 * 128, CT * CH)),
                "Wo_t": _bf(wo.reshape(CT * 128, IT * 128)),
                "cosT": cos,
                "sinTs": sins,
            })

    trace = bool(int(os.environ.get("KERNEL_TRACE", "0")))
    res = bass_utils.run_bass_kernel_spmd(
        nc, in_maps, core_ids=list(range(8)), trace=trace)
    last_result = res

    out = np.empty((B, T, C), dtype=np.float32)
    for b in range(B):
        acc = res.results[2 * b]["outT"].astype(np.float32) + \
            res.results[2 * b + 1]["outT"].astype(np.float32)
        out[b] = acc.T
    return out
